# revision 26
# baseline (speedup 1.0000x reference)
"""Trainium2 Bass kernel: batch-invariant causal multi-head attention.

Sharding (8 NeuronCores): core c owns batch c//4 and head group c%4 (4 of 16
heads = 256 of 1024 features). Wq/Wk/Wv are split column-wise by head group,
Wo row-wise; each core streams only its batch's query/key/value (pre-transposed
on host to [E, S] fp8e4m3 so every DMA is contiguous).

Per core (chunk-pipelined: project chunk j -> attention chunk j -> Wo chunk j):
  - Projections run as fp8 DoubleRow matmuls (0.5 PE cycles/row, contraction
    256 per instruction): x fp8 moving, W fp8 stationary, f32 PSUM accumulate,
    bias added on DVE into f16 q/k planes; V written fp8 with an appended
    fp8 ones column so softmax denominators fall out of the PV matmul chain.
  - scoresT = K_tile^T . Q in [k, q] layout (f16 operands, 1 cyc/row); no
    max-shift (softmax is shift-invariant; a fixed -2 shift inside exp keeps
    exp(s) within fp8e4m3 range). Diagonal tiles add a stairstep -1e9 mask
    via an identity-lhsT matmul before exp.
  - exp on ScalarE only (everything else is kept off ACT): below-diagonal
    k-tiles exp to fp8 p-planes grouped in k-tile PAIRS so PV runs as fp8
    DoubleRow over 256 k-positions per instruction; diagonal tiles exp to
    f16 and run PV per-tile (f16 moving).
  - normalize: DVE reciprocal of the denominator row, K=1 ones matmul
    broadcasts it across partitions, Pool multiplies into fp8 a-planes.
  - Wo: one fp8 DoubleRow matmul per 128-feature output block (contraction
    256 over both a-planes), PSUM converted to f16 on Pool/DVE, DMA'd out as
    f16 partials [E, S]; host sums 4 partials per batch, transposes, adds
    the constant Wo@bv + bo (attention rows sum to 1).
"""

import sys

if "/opt/trn_rl_repo" not in sys.path:
    sys.path.insert(0, "/opt/trn_rl_repo")

import numpy as np

S, B, E, H, D, P = 2048, 2, 1024, 16, 64, 128
NCORES = 8
CHUNK = 512               # q-chunk / matmul moving free dim
NJ = S // CHUNK           # 4 q-chunks
NT = S // P               # 16 k-tiles
ET = E // P               # 8 e-tiles (contraction)
NEG = -30000.0            # f16-finite mask value (scaled: exp(-3750) -> 0)
EXP_SHIFT = -4.0          # exp(s*scale + shift): keeps exp within fp8e4m3 range

_cache = {}


def _build_program():
    import concourse.tile as tile
    import concourse.mybir as mybir
    from concourse import bacc

    f32 = mybir.dt.float32
    f32r = mybir.dt.float32r
    f16 = mybir.dt.float16
    f8 = mybir.dt.float8e4
    AF = mybir.ActivationFunctionType
    DR = mybir.MatmulPerfMode.DoubleRow

    nc = bacc.Bacc("TRN2", target_bir_lowering=False, debug=False)

    qt = nc.dram_tensor("qt", [E, S], f16, kind="ExternalInput").ap()
    kt = nc.dram_tensor("kt", [E, S], f16, kind="ExternalInput").ap()
    vt = nc.dram_tensor("vt", [E, S], f16, kind="ExternalInput").ap()
    # wq/wk: [p(contraction-within-tile), e-tile, out-plane, out]
    wq = nc.dram_tensor("wq", [P, ET, 2, P], f16, kind="ExternalInput").ap()
    wk = nc.dram_tensor("wk", [P, ET, 2, P], f16, kind="ExternalInput").ap()
    # wv moving: [p, e-tile, out-plane, 128 features]
    wv = nc.dram_tensor("wv", [P, ET, 2, P], f16, kind="ExternalInput").ap()
    wo = nc.dram_tensor("wo", [P, ET, 2, P], f16, kind="ExternalInput").ap()
    bqk = nc.dram_tensor("bqk", [P, 2, 2], f32, kind="ExternalInput").ap()
    maskblk = nc.dram_tensor("maskblk", [P, 2 * P], f16, kind="ExternalInput").ap()
    identr = nc.dram_tensor("identr", [P, P], f16, kind="ExternalInput").ap()
    onesr = nc.dram_tensor("onesr", [1, P], f32r, kind="ExternalInput").ap()
    outp = nc.dram_tensor("outp", [ET, P, S], f16, kind="ExternalOutput").ap()

    scale = 1.0 / np.sqrt(D)

    with tile.TileContext(nc) as tc:
        with (
            tc.tile_pool(name="const", bufs=1) as cpool,
            tc.tile_pool(name="persist", bufs=1) as perst,
            tc.tile_pool(name="xin", bufs=9) as xin,
            tc.tile_pool(name="p8t", bufs=6) as p8pool,
            tc.tile_pool(name="p16t", bufs=4) as p16pool,
            tc.tile_pool(name="recip", bufs=4) as rpool,
            tc.tile_pool(name="osb", bufs=2) as osb,
            tc.tile_pool(name="outsb", bufs=3) as outsb,
            # PSUM budget (8 banks): mm 2x1 + s2 2x2 + pv 2x1 = 8
            tc.tile_pool(name="ps_mm", bufs=2, space="PSUM") as ps_mm,
            tc.tile_pool(name="ps_s", bufs=2, space="PSUM") as ps_s,
            tc.tile_pool(name="ps_pv", bufs=2, space="PSUM") as ps_pv,
        ):
            # ---- constants (wk first: proj k j=0 is the first consumer) ----
            wq_sb = cpool.tile([P, ET, 2, P], f16, tag="wq")
            wk_sb = cpool.tile([P, ET, 2, P], f16, tag="wk")
            wv_sb = cpool.tile([P, ET, 2, P], f16, tag="wv")
            wo_sb = cpool.tile([P, ET, 2, P], f16, tag="wo")
            bqk_sb = cpool.tile([P, 2, 2], f32, tag="bqk")
            mask_sb = cpool.tile([P, 2 * P], f16, tag="maskblk")
            idr_sb = cpool.tile([P, P], f16, tag="identr")
            ones_sb = cpool.tile([1, P], f32r, tag="ones")
            shift_sb = cpool.tile([P, 1], f32, tag="shift")
            nc.gpsimd.memset(shift_sb[:], EXP_SHIFT)
            nc.scalar.dma_start(wk_sb[:], wk[:])
            nc.scalar.dma_start(bqk_sb[:], bqk[:])
            nc.scalar.dma_start(wq_sb[:], wq[:])
            nc.scalar.dma_start(wv_sb[:], wv[:])
            nc.scalar.dma_start(idr_sb[:], identr[:])
            nc.scalar.dma_start(mask_sb[:], maskblk[:])
            nc.scalar.dma_start(ones_sb[:], onesr[:])
            nc.scalar.dma_start(wo_sb[:], wo[:])

            # warm the ACT exp table during the DMA-bound startup
            warm = rpool.tile([P, 2], f32, tag="warm", name="warm")
            nc.scalar.activation(warm[:], bqk_sb[:, 0, :], AF.Exp, scale=1.0)

            # ---- persistent activations ----
            q_sb = perst.tile([P, 2, S], f16, tag="q")
            k_sb = perst.tile([P, 2, S], f16, tag="k")
            a_sb = perst.tile([P, 2, S], f16, tag="attnT")
            # V twice: f16 for diagonal PV (short rows see V unaveraged),
            # fp8 for below-diagonal DoubleRow PV (noise averages out)
            v16_sb = perst.tile([P, NT, 260], f16, tag="v16")
            v8_sb = perst.tile([P, NT, 260], f8, tag="v8")
            for hl in range(4):
                on = slice(65 * hl + 64, 65 * hl + 65)
                nc.gpsimd.memset(v16_sb[:, :, on], 1.0)
                nc.gpsimd.memset(v8_sb[:, :, on], 1.0)

            def proj_steps(which, j):
                w_sb, x_ap, bi = {
                    "q": (wq_sb, qt, 0),
                    "k": (wk_sb, kt, 1),
                }[which]
                cs = slice(CHUNK * j, CHUNK * (j + 1))
                ps0 = ps_mm.tile([P, CHUNK], f32, tag="mm")
                ps1 = ps_mm.tile([P, CHUNK], f32, tag="mm")
                gw = 1 if (which == "k" and j == 0) else 4
                for tg in range(ET // gw):
                    xt = xin.tile([P, 4, CHUNK], f16, tag="xin")
                    nc.sync.dma_start(
                        xt[:, :gw, :],
                        x_ap[gw * P * tg:gw * P * (tg + 1), cs]
                        .rearrange("(o p) s -> p o s", p=P))
                    for o in range(gw):
                        t = gw * tg + o
                        nc.tensor.matmul(ps0[:], w_sb[:, t, 0, :], xt[:, o, :],
                                         start=(t == 0), stop=(t == ET - 1))
                        nc.tensor.matmul(ps1[:], w_sb[:, t, 1, :], xt[:, o, :],
                                         start=(t == 0), stop=(t == ET - 1))
                        yield
                dst = q_sb if which == "q" else k_sb
                for dtp, ps in ((0, ps0), (1, ps1)):
                    nc.vector.tensor_scalar_add(
                        dst[:, dtp, cs], ps[:], bqk_sb[:, dtp, bi:bi + 1])

            def proj_v_steps(j):
                # swapped operands: xt stationary, f16 weights moving ->
                # v in [s, d] layout directly; f16 copy on DVE, fp8 on Pool.
                cs = slice(CHUNK * j, CHUNK * (j + 1))
                xts = []
                for tg in range(2):
                    xt = xin.tile([P, 4, CHUNK], f16, tag="xin")
                    nc.sync.dma_start(
                        xt[:],
                        vt[4 * P * tg:4 * P * (tg + 1), cs]
                        .rearrange("(o p) s -> p o s", p=P))
                    xts.append(xt)
                for si in range(CHUNK // P):
                    kt_idx = (CHUNK // P) * j + si
                    psv = ps_mm.tile([P, 2 * P], f32, tag="mm", name="psv")
                    for t in range(ET):
                        nc.tensor.matmul(
                            psv[:],
                            xts[t // 4][:, t % 4, P * si:P * (si + 1)],
                            wv_sb[:, t, :, :],
                            start=(t == 0), stop=(t == ET - 1))
                    v16d = v16_sb[:, kt_idx, :].rearrange(
                        "p (h x) -> p h x", x=65)[:, :, 0:64]
                    nc.vector.tensor_copy(
                        v16d, psv[:].rearrange("p (h x) -> p h x", x=64))
                    with nc.allow_low_precision(reason="fp8 V for DoubleRow PV"):
                        nc.gpsimd.tensor_copy(
                            v8_sb[:, kt_idx, :].rearrange(
                                "p (h x) -> p h x", x=65)[:, :, 0:64], v16d)
                    yield

            # Schraudolph exp on DVE: bitcast_f16(int16(s*1024*log2e*scale
            # + 1024*(15 + log2e*EXP_SHIFT) - 38.5)) ~ exp(s*scale+EXP_SHIFT)
            SCHR_A = 1024.0 * 1.4426950408889634 * scale
            SCHR_B = 1024.0 * (15.0 + 1.4426950408889634 * EXP_SHIFT) - 38.5

            def attn_steps(dt, j, pend):
                # both heads of plane dt; q-chunk j; k-tile pairs DoubleRow.
                # PV matmuls are emitted ~2 tiles behind their exp so the
                # in-order PE queue never parks on an unfinished exp; `pend`
                # is shared across planes so one plane's tail PVs+normalize
                # overlap the next plane's score matmuls.
                cs0 = CHUNK * j
                csl = slice(cs0, cs0 + CHUNK)
                pvo = [ps_pv.tile([65, CHUNK], f32, tag="pv", name=f"pv{_h}")
                       for _h in range(2)]
                first = [True]

                def flush(n):
                    while len(pend) > n:
                        pend.pop(0)()

                def mk_pv_dr(tp, p8):
                    def go():
                        for hh in range(2):
                            hl = 2 * dt + hh
                            nc.tensor.matmul(
                                pvo[hh][:], v8_sb[:, 2 * tp:2 * tp + 2,
                                                  65 * hl:65 * hl + 65],
                                p8[:, hh, :, :],
                                start=first[0], stop=False, perf_mode=DR)
                        first[0] = False
                    return go

                def mk_pv_f16(t, p16v, u):
                    def go():
                        for hh in range(2):
                            hl = 2 * dt + hh
                            nc.tensor.matmul(
                                pvo[hh][:], v16_sb[:, t, 65 * hl:65 * hl + 65],
                                p16v[:, hh, u, :],
                                start=first[0], stop=False)
                        first[0] = False
                    return go

                def mk_pv_diag(t, p16, r, i):
                    def go():
                        for hh in range(2):
                            hl = 2 * dt + hh
                            nc.tensor.matmul(
                                pvo[hh][:, r:CHUNK],
                                v16_sb[:, t, 65 * hl:65 * hl + 65],
                                p16[:, hh, r:],
                                start=first[0], stop=(i == CHUNK // P - 1))
                        first[0] = False
                    return go

                # below-diagonal k-tile pairs; exp on ACT (fp8 p, DoubleRow
                # PV) or offloaded to DVE (Schraudolph f16 p, f16 PV) in the
                # later, projection-free phases
                for tp in range(2 * j):
                    dve = j == NJ - 1 and tp % 3 == 2
                    if dve:
                        pt = p16pool.tile([P, 2, 2, CHUNK], f16, tag="p16p",
                                          name="p16p")
                    else:
                        pt = p8pool.tile([P, 2, 2, CHUNK], f8, tag="p8",
                                         name="p8")
                    for u in range(2):
                        t = 2 * tp + u
                        s2 = ps_s.tile([P, 2, CHUNK], f32, tag="s2", name="s2")
                        for hh in range(2):
                            hs = slice(64 * hh, 64 * hh + 64)
                            nc.tensor.matmul(
                                s2[:, hh, :],
                                k_sb[hs, dt, P * t:P * (t + 1)],
                                q_sb[hs, dt, csl],
                                start=True, stop=True)
                        if dve:
                            with nc.allow_low_precision(reason="schraudolph"):
                                nc.vector.tensor_scalar(
                                    pt[:, :, u, :].bitcast(mybir.dt.int16),
                                    s2[:], SCHR_A, SCHR_B,
                                    op0=mybir.AluOpType.mult,
                                    op1=mybir.AluOpType.add)
                            pend.append(mk_pv_f16(t, pt, u))
                        else:
                            nc.scalar.activation(
                                pt[:, :, u, :], s2[:], AF.Exp,
                                scale=scale, bias=shift_sb[:])
                        flush(2)
                        yield
                    if not dve:
                        pend.append(mk_pv_dr(tp, pt))
                # diagonal tiles (stairstep mask, f16 p, f16 V, per-tile PV)
                for i in range(CHUNK // P):
                    t = (CHUNK // P) * j + i
                    r = P * i
                    s2 = ps_s.tile([P, 2, CHUNK], f32, tag="s2", name="s2d")
                    for hh in range(2):
                        hs = slice(64 * hh, 64 * hh + 64)
                        nc.tensor.matmul(
                            s2[:, hh, r:CHUNK],
                            k_sb[hs, dt, P * t:P * (t + 1)],
                            q_sb[hs, dt, cs0 + r:cs0 + CHUNK],
                            start=True, stop=False)
                    nc.tensor.matmul(
                        s2[:, :, r:r + P],
                        idr_sb[:],
                        mask_sb[:].rearrange("p (h x) -> p h x", h=2),
                        start=False, stop=True)
                    p16 = p16pool.tile([P, 2, CHUNK], f16, tag="p16")
                    nc.scalar.activation(
                        p16[:, :, r:], s2[:, :, r:], AF.Exp,
                        scale=scale, bias=shift_sb[:])
                    pend.append(mk_pv_diag(t, p16, r, i))
                    flush(2)
                    yield

                def normalize():
                    # normalize -> f16 a-planes (one PSUM operand max per
                    # tensor_tensor: numerator goes via an SBUF copy)
                    for hh in range(2):
                        hs = slice(64 * hh, 64 * hh + 64)
                        rc = rpool.tile([1, CHUNK], f32r, tag="recip")
                        with nc.allow_low_precision(reason="feeds f32r matmul"):
                            nc.vector.reciprocal(rc[:], pvo[hh][64:65, :])
                        o_t = osb.tile([64, CHUNK], f32, tag="o", name="o_t")
                        nc.vector.tensor_copy(o_t[:], pvo[hh][0:64, :])
                        bc = ps_s.tile([64, CHUNK], f32, tag="s2", name="bc")
                        nc.tensor.matmul(bc[:], ones_sb[:, 0:64], rc[:],
                                         start=True, stop=True)
                        nc.vector.tensor_tensor(
                            a_sb[hs, dt, csl], o_t[:], bc[:],
                            op=mybir.AluOpType.mult)
                pend.append(normalize)

            def wo_chunk(j):
                cs = slice(CHUNK * j, CHUNK * (j + 1))
                out_eng = nc.sync if j == NJ - 1 else nc.scalar
                ow = 2 if j == NJ - 1 else 4
                for tg in range(ET // ow):
                    ot = outsb.tile([P, 4, CHUNK], f16, tag="out")
                    for o in range(ow):
                        t = ow * tg + o
                        wops = ps_pv.tile([P, CHUNK], f32, tag="pv", name="wops")
                        nc.tensor.matmul(wops[:], wo_sb[:, t, 0, :],
                                         a_sb[:, 0, cs], start=True, stop=False)
                        nc.tensor.matmul(wops[:], wo_sb[:, t, 1, :],
                                         a_sb[:, 1, cs], start=False, stop=True)
                        nc.vector.tensor_copy(ot[:, o, :], wops[:])
                    out_eng.dma_start(
                        outp[ow * tg:ow * (tg + 1), :, cs]
                        .rearrange("o p s -> p o s"), ot[:, :ow, :])

            def drain(gen, n=1 << 30):
                for _ in range(n):
                    if next(gen, _SENTINEL) is _SENTINEL:
                        return True
                return False

            _SENTINEL = object()
            from itertools import chain as _chain

            # prologue: project q/k of chunk 0 (DMA-bound startup)
            drain(proj_steps("k", 0))
            drain(proj_steps("q", 0))
            # steady state: attention(j) paced by ScalarE exp; interleave the
            # next chunk's projection matmuls so the PE never starves
            for j in range(NJ):
                parts = [] if j else [proj_v_steps(0)]
                if j < NJ - 1:
                    parts += [proj_steps("k", j + 1), proj_steps("q", j + 1),
                              proj_v_steps(j + 1)]
                filler = _chain(*parts)
                pend = []
                main = _chain(attn_steps(0, j, pend), attn_steps(1, j, pend))
                ratio = (3, 2, 1, 1)[j]
                for _ in main:
                    drain(filler, ratio)
                while pend:
                    pend.pop(0)()
                drain(filler)
                wo_chunk(j)

    nc.compile()
    return nc


def _host_prep(query, key, value, Wq, bq, Wk, bk, Wv, bv, Wo, bo):
    import ml_dtypes
    f8 = ml_dtypes.float8_e4m3

    qt = np.ascontiguousarray(np.asarray(query, np.float32).transpose(1, 2, 0)).astype(np.float16)
    kt = np.ascontiguousarray(np.asarray(key, np.float32).transpose(1, 2, 0)).astype(np.float16)
    vt = np.ascontiguousarray(np.asarray(value, np.float32).transpose(1, 2, 0)).astype(np.float16)
    mb = np.where(np.arange(P)[None, :] >= np.arange(P)[:, None],
                  0.0, NEG).astype(np.float16)
    maskblk = np.concatenate([mb, mb], axis=1)
    ident = np.eye(P, dtype=np.float16)
    Wq, Wk, Wv, Wo = (np.asarray(a, np.float32) for a in (Wq, Wk, Wv, Wo))
    bq, bk = (np.asarray(a, np.float32) for a in (bq, bk))
    in_maps = []
    for c in range(NCORES):
        b, g = c // 4, c % 4
        F = slice(256 * g, 256 * (g + 1))
        # wq/wk/wv [p(e-within-tile), e-tile, out-plane, out-feature]
        wq_l = Wq[F, :].T.reshape(ET, P, 2, P).transpose(1, 0, 2, 3)
        wk_l = Wk[F, :].T.reshape(ET, P, 2, P).transpose(1, 0, 2, 3)
        wv_l = Wv[F, :].T.reshape(ET, P, 2, P).transpose(1, 0, 2, 3)
        # wo [p, t, dt, c] row-slice of Wo for this core's 256 features
        wo_l = Wo[:, F].T.reshape(2, P, ET, P).transpose(1, 2, 0, 3)
        in_maps.append({
            "qt": qt[b], "kt": kt[b], "vt": vt[b],
            "wq": np.ascontiguousarray(wq_l).astype(np.float16),
            "wk": np.ascontiguousarray(wk_l).astype(np.float16),
            "wv": np.ascontiguousarray(wv_l).astype(np.float16),
            "wo": np.ascontiguousarray(wo_l).astype(np.float16),
            "bqk": np.ascontiguousarray(np.stack(
                [bq[F].reshape(2, P).T, bk[F].reshape(2, P).T], axis=2)),
            "maskblk": maskblk, "identr": ident,
            "onesr": np.ones((1, P), np.float32),
        })
    return in_maps


def _get_runner():
    """Build the program once and wrap it in a jit-compiled 8-core SPMD
    executable that is reused across kernel() calls."""
    if "runner" in _cache:
        return _cache["runner"]

    import jax
    from jax.sharding import Mesh, PartitionSpec
    try:
        from jax.experimental.shard_map import shard_map
    except ImportError:
        from jax import shard_map
    import concourse.mybir as mybir
    import concourse.bass2jax as b2j

    nc = _cache.get("nc") or _build_program()
    _cache["nc"] = nc
    b2j.install_neuronx_cc_hook()

    in_names, out_names, out_avals, out_shapes = [], [], [], []
    for alloc in nc.m.functions[0].allocations:
        if not isinstance(alloc, mybir.MemoryLocationSet):
            continue
        name = alloc.memorylocations[0].name
        if alloc.kind == "ExternalInput":
            if nc.partition_id_tensor is None or name != nc.partition_id_tensor.name:
                in_names.append(name)
        elif alloc.kind == "ExternalOutput":
            out_names.append(name)
            shape = tuple(alloc.tensor_shape)
            dtype = mybir.dt.np(alloc.dtype)
            out_avals.append(jax.core.ShapedArray(shape, dtype))
            out_shapes.append((shape, dtype))
    n_params = len(in_names)
    all_in = list(in_names) + out_names
    pid_name = nc.partition_id_tensor.name if nc.partition_id_tensor else None
    if pid_name is not None:
        all_in.append(pid_name)

    def _body(*args):
        ops = list(args)
        if pid_name is not None:
            ops.append(b2j.partition_id_tensor())
        outs = b2j._bass_exec_p.bind(
            *ops, out_avals=tuple(out_avals), in_names=tuple(all_in),
            out_names=tuple(out_names), lowering_input_output_aliases=(),
            sim_require_finite=True, sim_require_nnan=True, nc=nc)
        return tuple(outs)

    devices = jax.devices()[:NCORES]
    mesh = Mesh(np.asarray(devices), ("core",))
    nio = n_params + len(out_names)
    sharded = jax.jit(
        shard_map(_body, mesh=mesh, in_specs=(PartitionSpec("core"),) * nio,
                  out_specs=(PartitionSpec("core"),) * len(out_names),
                  check_rep=False),
        donate_argnums=tuple(range(n_params, nio)), keep_unused=True)

    def run(in_maps):
        concat_in = [
            np.concatenate([np.asarray(in_maps[c][n]) for c in range(NCORES)], axis=0)
            for n in in_names]
        zeros = [np.zeros((NCORES * s[0], *s[1:]), d) for s, d in out_shapes]
        out_arrs = sharded(*concat_in, *zeros)
        return [
            {name: np.asarray(out_arrs[i]).reshape(NCORES, *out_shapes[i][0])[c]
             for i, name in enumerate(out_names)}
            for c in range(NCORES)]

    _cache["runner"] = run
    return run


def kernel(query, key, value, Wq, bq, Wk, bk, Wv, bv, Wo, bo):
    in_maps = _host_prep(query, key, value, Wq, bq, Wk, bk, Wv, bv, Wo, bo)

    results = None
    last_exc = None
    for attempt in range(3):
        try:
            results = _get_runner()(in_maps)
            break
        except Exception as exc:  # transient NRT/device wedges: rebuild + retry
            last_exc = exc
            _cache.pop("runner", None)
    if results is None:
        from concourse.bass_utils import run_bass_kernel_spmd
        nc = _cache.get("nc") or _build_program()
        _cache["nc"] = nc
        try:
            results = run_bass_kernel_spmd(
                nc, in_maps, core_ids=list(range(NCORES))).results
        except Exception:
            raise last_exc

    out = np.empty((S, B, E), np.float32)
    for b in range(B):
        acc = np.zeros((E, S), np.float64)
        for g in range(4):
            acc += results[4 * b + g]["outp"].reshape(E, S).astype(np.float64)
        out[:, b, :] = acc.T
    # attn rows sum to 1, so the V bias contributes the constant Wo @ bv
    const = (np.asarray(Wo, np.float64) @ np.asarray(bv, np.float64)
             + np.asarray(bo, np.float64)).astype(np.float32)
    return out + const
